# revision 8
# baseline (speedup 1.0000x reference)
"""Conv2d 3x3 (stride 1, pad 1) as implicit GEMM on 8 Trainium2 NeuronCores.

Problem: x [32,128,56,56] f32, weight [256,128,3,3] f32, bias [256] f32
         -> out [32,256,56,56] f32.

Sharding: data-parallel over batch. Each of the 8 cores gets 4 images;
weight/bias are replicated. No collectives; outputs are concatenated on host.

Per-core kernel (implicit GEMM, bf16 matmuls, fp32 PSUM accumulation):
  - x is host-padded + bf16-cast to [4,128,58,58]; each image's padded plane
    lives in SBUF as a [128, 58, 58] tile (in-channels on partitions).
  - weight is host-rearranged to [128, 9, 256] bf16 (in-ch partitions, 3x3
    taps, out-ch free) so lhsT slices need no on-device transpose.
  - For each image and band of 8 output rows (7 bands), then out-channel
    group g (2 groups of 128): accumulate 9 matmuls (one per tap) into a
    [128, 448] PSUM tile: psum += W[:, ki, g*128:...].T @ xpad[:, rows+kh, kw:kw+56]
    (band-outer / group-inner so each input chunk feeds two bands' worth of
    matmuls -- halves the input DMA arrival-rate requirement in the head).
  - bias-add + PSUM->SBUF(bf16) on the scalar engine, then DMA to DRAM;
    host casts back to fp32. bf16 output halves the output traffic; total
    rel err ~2.5e-3 vs the 2e-2 gate.

Performance notes (measured on trn2 via NTFF/perfetto):
  - fp32r was LDWEIGHTS-bound (~213ns/MM): fp32 weights can't use Fast
    Weight Load and every fp32r matmul self-loads its weights. bf16 enables
    FWL; the weight load (~97ns) hides under the 448-col stream (186.7ns)
    -> ~190ns/MM cadence. bf16 inputs keep fp32 PSUM accumulation; rel err
    ~2e-3.
  - Run-to-run chip state (P0 power level / co-tenancy) can downclock the
    PE 2.4->2.0 GHz, scaling the whole MM span by 1.2x. Nothing
    kernel-side controls this.
  - Head: NEFF preamble ~7.3us (fixed), then memset-fed warm-up matmuls
    keep the PE busy so the HAM clock-gate hits 8/8 by ~11us; real matmuls
    start as soon as the first input chunk lands (~8.5us).
  - Weight DMAs are batched (tap0-g0 tiny, taps1-8-g0, all-g1) because the
    Sync engine issues DMA descriptors serially at ~610ns each -- 18
    separate weight DMAs would starve the head.
"""

import numpy as np
import ml_dtypes

import concourse.bacc as bacc
import concourse.mybir as mybir
import concourse.tile as tile
from concourse.bass_utils import run_bass_kernel_spmd

N_CORES = 8
B, C_IN, H, W = 32, 128, 56, 56
C_OUT = 256
KH = KW = 3
B_LOC = B // N_CORES          # 4 images per core
HP, WP = H + 2, W + 2         # 58 (pad=1)
ROWS = 8                      # output rows per matmul
NCHUNK = H // ROWS            # 7 bands
NFREE = ROWS * W              # 448 = matmul free dim (fits one PSUM bank)
NGRP = C_OUT // 128           # 2 out-channel groups

MM_DT = mybir.dt.bfloat16
NP_BF16 = ml_dtypes.bfloat16


def _build():
    nc = bacc.Bacc(None, target_bir_lowering=False)
    xp = nc.dram_tensor("xp", [B_LOC, C_IN, HP, WP], MM_DT, kind="ExternalInput")
    wt = nc.dram_tensor("wt", [C_IN, KH * KW, C_OUT], MM_DT, kind="ExternalInput")
    bz = nc.dram_tensor("bz", [128, NGRP], mybir.dt.float32, kind="ExternalInput")
    out = nc.dram_tensor(
        "out", [B_LOC, NGRP, 128, H * W], MM_DT, kind="ExternalOutput"
    )

    with tile.TileContext(nc) as tc:
        with (
            tc.tile_pool(name="const", bufs=1) as cpool,
            tc.tile_pool(name="xin", bufs=B_LOC) as xpool,
            tc.tile_pool(name="oout", bufs=6) as opool,
            tc.tile_pool(name="psum", bufs=4, space="PSUM") as pspool,
        ):
            # PE warm-up: dummy bf16 matmuls on a memset tile -- no DMA
            # dependency, so the PE is busy right after the NEFF preamble
            # and the HAM clock-gate (1.2 -> 2.4 GHz) fires early. Kept
            # short so real matmuls start as soon as input lands.
            wu = cpool.tile([128, NFREE], MM_DT)
            nc.vector.memset(wu[:], 1.0)
            wu_ps = pspool.tile([128, NFREE], mybir.dt.float32, tag="warm", bufs=1)
            n_warm = 7
            for i in range(n_warm):
                nc.tensor.matmul(
                    wu_ps[:],
                    wu[:, 0:128],
                    wu[:],
                    start=(i == 0),
                    stop=(i == n_warm - 1),
                )

            w_tile = cpool.tile([C_IN, KH * KW, C_OUT], MM_DT)
            b_tile = cpool.tile([128, NGRP], mybir.dt.float32)
            x_tiles = [
                xpool.tile([C_IN, HP, WP], MM_DT, name=f"x_img{b}", tag="ximg")
                for b in range(B_LOC)
            ]

            # chunk rc of image b: band-aligned row ranges. Band rc needs
            # padded rows [rc*ROWS, rc*ROWS+ROWS+2); chunk 0 covers rows
            # 0..9, chunk rc>=1 adds rows rc*ROWS+2 .. rc*ROWS+9.
            def load_chunk(b, rc):
                lo = 0 if rc == 0 else rc * ROWS + 2
                hi = rc * ROWS + ROWS + 2
                nc.sync.dma_start(x_tiles[b][:, lo:hi], xp[b, :, lo:hi])

            # Head DMAs on TWO issue pipelines: DMA descriptors cost ~610ns
            # each to issue and land ~2us later, so weights go out on the
            # (otherwise idle until ~12.6us) Scalar hwdge queue while input
            # chunks go out on Sync -- everything lands before its first
            # consuming matmul.
            nc.scalar.dma_start(w_tile[:, 0, 0:128], wt[:, 0, 0:128])
            nc.scalar.dma_start(w_tile[:, 1:3, 0:128], wt[:, 1:3, 0:128])
            nc.scalar.dma_start(w_tile[:, 3:, 0:128], wt[:, 3:, 0:128])
            nc.scalar.dma_start(w_tile[:, :, 128:256], wt[:, :, 128:256])
            nc.scalar.dma_start(b_tile[:], bz[:])
            for rc in range(NCHUNK):
                load_chunk(0, rc)

            def band(b, g, rc, r0, nrows):
                """One accumulation group: rows [r0, r0+nrows) of image b,
                out-channel group g; bias-add and store."""
                nf = nrows * W
                ps = pspool.tile([128, NFREE], mybir.dt.float32, tag="ps", bufs=5)
                for ki in range(KH * KW):
                    kh, kw = divmod(ki, KW)
                    nc.tensor.matmul(
                        ps[:, 0:nf],
                        w_tile[:, ki, g * 128 : (g + 1) * 128],
                        x_tiles[b][:, r0 + kh : r0 + kh + nrows, kw : kw + W],
                        start=(ki == 0),
                        stop=(ki == KH * KW - 1),
                    )
                o_tile = opool.tile(
                    [128, NFREE], MM_DT, name=f"o_{b}_{g}_{r0}", tag="ot"
                )
                nc.scalar.activation(
                    o_tile[:, 0:nf],
                    ps[:, 0:nf],
                    mybir.ActivationFunctionType.Identity,
                    bias=b_tile[:, g : g + 1],
                    scale=1.0,
                )
                nc.sync.dma_start(
                    out[b, g, :, r0 * W : r0 * W + nf], o_tile[:, 0:nf]
                )

            for b in range(B_LOC):
                for rc in range(NCHUNK):
                    # trickle next image's chunks so prefetch doesn't starve
                    # this image's output DMAs
                    if b + 1 < B_LOC:
                        load_chunk(b + 1, rc)
                    for g in range(NGRP):
                        last = (
                            b == B_LOC - 1 and rc == NCHUNK - 1 and g == NGRP - 1
                        )
                        if last:
                            # split the final band so the tail ACT+DMA chain
                            # after the last matmul is half as long
                            band(b, g, rc, rc * ROWS, ROWS // 2)
                            band(b, g, rc, rc * ROWS + ROWS // 2, ROWS // 2)
                        else:
                            band(b, g, rc, rc * ROWS, ROWS)
    nc.finalize()
    return nc


_NC = None


def _prep_inputs(x, weight, bias):
    x = np.asarray(x, dtype=np.float32)
    weight = np.asarray(weight, dtype=np.float32)
    bias = np.asarray(bias, dtype=np.float32)
    xp = np.zeros((B, C_IN, HP, WP), dtype=NP_BF16)
    xp[:, :, 1 : H + 1, 1 : W + 1] = x.astype(NP_BF16)
    # wt[p, kh*3+kw, o] = weight[o, p, kh, kw]
    wt = np.ascontiguousarray(
        weight.transpose(1, 2, 3, 0).reshape(C_IN, KH * KW, C_OUT).astype(NP_BF16)
    )
    # bz[p, g] = bias[g*128 + p]
    bz = np.ascontiguousarray(bias.reshape(NGRP, 128).T)
    return xp, wt, bz


def kernel(x, weight, bias, trace=False):
    global _NC
    xp, wt, bz = _prep_inputs(x, weight, bias)
    if _NC is None:
        _NC = _build()
    in_maps = [
        {"xp": xp[c * B_LOC : (c + 1) * B_LOC], "wt": wt, "bz": bz}
        for c in range(N_CORES)
    ]
    res = run_bass_kernel_spmd(
        _NC, in_maps, core_ids=list(range(N_CORES)), trace=trace
    )
    outs = [
        r["out"].astype(np.float32).reshape(B_LOC, C_OUT, H, W)
        for r in res.results
    ]
    full = np.concatenate(outs, axis=0)
    if trace:
        return full, res
    return full


# revision 9
# speedup vs baseline: 1.0220x; 1.0220x over previous
"""Conv2d 3x3 (stride 1, pad 1) as implicit GEMM on 8 Trainium2 NeuronCores.

Problem: x [32,128,56,56] f32, weight [256,128,3,3] f32, bias [256] f32
         -> out [32,256,56,56] f32.

Sharding: data-parallel over batch. Each of the 8 cores gets 4 images;
weight/bias are replicated. No collectives; outputs are concatenated on host.

Per-core kernel (implicit GEMM, bf16 matmuls, fp32 PSUM accumulation):
  - x is host-padded + bf16-cast to [4,128,58,58]; each image's padded plane
    lives in SBUF as a [128, 58, 58] tile (in-channels on partitions).
  - weight is host-rearranged to [128, 9, 256] bf16 (in-ch partitions, 3x3
    taps, out-ch free) so lhsT slices need no on-device transpose.
  - For each image and band of 8 output rows (7 bands), then out-channel
    group g (2 groups of 128): accumulate 9 matmuls (one per tap) into a
    [128, 448] PSUM tile: psum += W[:, ki, g*128:...].T @ xpad[:, rows+kh, kw:kw+56]
    (band-outer / group-inner so each input chunk feeds two bands' worth of
    matmuls -- halves the input DMA arrival-rate requirement in the head).
  - bias-add + PSUM->SBUF(bf16) on the scalar engine, then DMA to DRAM;
    host casts back to fp32. bf16 output halves the output traffic; total
    rel err ~2.5e-3 vs the 2e-2 gate.

Performance notes (measured on trn2 via NTFF/perfetto):
  - fp32r was LDWEIGHTS-bound (~213ns/MM): fp32 weights can't use Fast
    Weight Load and every fp32r matmul self-loads its weights. bf16 enables
    FWL; the weight load (~97ns) hides under the 448-col stream (186.7ns)
    -> ~190ns/MM cadence. bf16 inputs keep fp32 PSUM accumulation; rel err
    ~2e-3.
  - Run-to-run chip state (P0 power level / co-tenancy) can downclock the
    PE 2.4->2.0 GHz, scaling the whole MM span by 1.2x. Nothing
    kernel-side controls this.
  - Head: NEFF preamble ~7.3us (fixed), then memset-fed warm-up matmuls
    keep the PE busy so the HAM clock-gate hits 8/8 by ~11us; real matmuls
    start as soon as the first input chunk lands (~8.5us).
  - Weight DMAs are batched (tap0-g0 tiny, taps1-8-g0, all-g1) because the
    Sync engine issues DMA descriptors serially at ~610ns each -- 18
    separate weight DMAs would starve the head.
"""

import numpy as np
import ml_dtypes

import concourse.bacc as bacc
import concourse.mybir as mybir
import concourse.tile as tile
from concourse.bass_utils import run_bass_kernel_spmd

N_CORES = 8
B, C_IN, H, W = 32, 128, 56, 56
C_OUT = 256
KH = KW = 3
B_LOC = B // N_CORES          # 4 images per core
HP, WP = H + 2, W + 2         # 58 (pad=1)
ROWS = 8                      # output rows per matmul
NCHUNK = H // ROWS            # 7 bands
NFREE = ROWS * W              # 448 = matmul free dim (fits one PSUM bank)
NGRP = C_OUT // 128           # 2 out-channel groups

MM_DT = mybir.dt.bfloat16
NP_BF16 = ml_dtypes.bfloat16


def _build():
    nc = bacc.Bacc(None, target_bir_lowering=False)
    xp = nc.dram_tensor("xp", [B_LOC, C_IN, HP, WP], MM_DT, kind="ExternalInput")
    wt = nc.dram_tensor("wt", [C_IN, KH * KW, C_OUT], MM_DT, kind="ExternalInput")
    bz = nc.dram_tensor("bz", [128, NGRP], mybir.dt.float32, kind="ExternalInput")
    out = nc.dram_tensor(
        "out", [B_LOC, NGRP, 128, H * W], MM_DT, kind="ExternalOutput"
    )

    with tile.TileContext(nc) as tc:
        with (
            tc.tile_pool(name="const", bufs=1) as cpool,
            tc.tile_pool(name="xin", bufs=B_LOC) as xpool,
            tc.tile_pool(name="oout", bufs=6) as opool,
            tc.tile_pool(name="psum", bufs=4, space="PSUM") as pspool,
        ):
            # PE warm-up: dummy bf16 matmuls on a memset tile -- no DMA
            # dependency, so the PE is busy right after the NEFF preamble
            # and the HAM clock-gate (1.2 -> 2.4 GHz) fires early. Kept
            # short so real matmuls start as soon as input lands.
            wu = cpool.tile([128, NFREE], MM_DT)
            nc.vector.memset(wu[:], 1.0)
            wu_ps = pspool.tile([128, NFREE], mybir.dt.float32, tag="warm", bufs=1)
            n_warm = 7
            for i in range(n_warm):
                nc.tensor.matmul(
                    wu_ps[:],
                    wu[:, 0:128],
                    wu[:],
                    start=(i == 0),
                    stop=(i == n_warm - 1),
                )

            w_tile = cpool.tile([C_IN, KH * KW, C_OUT], MM_DT)
            b_tile = cpool.tile([128, NGRP], mybir.dt.float32)
            x_tiles = [
                xpool.tile([C_IN, HP, WP], MM_DT, name=f"x_img{b}", tag="ximg")
                for b in range(B_LOC)
            ]

            # chunk rc of image b: band-aligned row ranges. Band rc needs
            # padded rows [rc*ROWS, rc*ROWS+ROWS+2); chunk 0 covers rows
            # 0..9, chunk rc>=1 adds rows rc*ROWS+2 .. rc*ROWS+9.
            def load_chunk(b, rc):
                lo = 0 if rc == 0 else rc * ROWS + 2
                hi = rc * ROWS + ROWS + 2
                nc.sync.dma_start(x_tiles[b][:, lo:hi], xp[b, :, lo:hi])

            # DMA priority order (Sync issues serially, ~610ns each, and
            # each queue's transfer lands ~2-3us after issue): chunk0 and
            # tap0-g0 feed the first matmul; taps1-2, taps3-8 and the g1
            # block are batched so they land before their first consuming
            # band without starving the chunk pipeline.
            load_chunk(0, 0)
            nc.sync.dma_start(w_tile[:, 0, 0:128], wt[:, 0, 0:128])
            nc.sync.dma_start(w_tile[:, 1:3, 0:128], wt[:, 1:3, 0:128])
            nc.sync.dma_start(w_tile[:, 3:, 0:128], wt[:, 3:, 0:128])
            nc.sync.dma_start(w_tile[:, :, 128:256], wt[:, :, 128:256])
            load_chunk(0, 1)
            nc.sync.dma_start(b_tile[:], bz[:])
            load_chunk(0, 2)
            load_chunk(0, 3)
            load_chunk(0, 4)
            load_chunk(0, 5)
            load_chunk(0, 6)

            def band(b, g, rc, r0, nrows):
                """One accumulation group: rows [r0, r0+nrows) of image b,
                out-channel group g; bias-add and store."""
                nf = nrows * W
                ps = pspool.tile([128, NFREE], mybir.dt.float32, tag="ps", bufs=5)
                for ki in range(KH * KW):
                    kh, kw = divmod(ki, KW)
                    nc.tensor.matmul(
                        ps[:, 0:nf],
                        w_tile[:, ki, g * 128 : (g + 1) * 128],
                        x_tiles[b][:, r0 + kh : r0 + kh + nrows, kw : kw + W],
                        start=(ki == 0),
                        stop=(ki == KH * KW - 1),
                    )
                o_tile = opool.tile(
                    [128, NFREE], MM_DT, name=f"o_{b}_{g}_{r0}", tag="ot"
                )
                nc.scalar.activation(
                    o_tile[:, 0:nf],
                    ps[:, 0:nf],
                    mybir.ActivationFunctionType.Identity,
                    bias=b_tile[:, g : g + 1],
                    scale=1.0,
                )
                nc.sync.dma_start(
                    out[b, g, :, r0 * W : r0 * W + nf], o_tile[:, 0:nf]
                )

            for b in range(B_LOC):
                for rc in range(NCHUNK):
                    # trickle next image's chunks so prefetch doesn't starve
                    # this image's output DMAs
                    if b + 1 < B_LOC:
                        load_chunk(b + 1, rc)
                    for g in range(NGRP):
                        last = (
                            b == B_LOC - 1 and rc == NCHUNK - 1 and g == NGRP - 1
                        )
                        if last:
                            # split the final band so the tail ACT+DMA chain
                            # after the last matmul is half as long
                            band(b, g, rc, rc * ROWS, ROWS // 2)
                            band(b, g, rc, rc * ROWS + ROWS // 2, ROWS // 2)
                        else:
                            band(b, g, rc, rc * ROWS, ROWS)
    nc.finalize()
    return nc


_NC = None


def _prep_inputs(x, weight, bias):
    x = np.asarray(x, dtype=np.float32)
    weight = np.asarray(weight, dtype=np.float32)
    bias = np.asarray(bias, dtype=np.float32)
    xp = np.zeros((B, C_IN, HP, WP), dtype=NP_BF16)
    xp[:, :, 1 : H + 1, 1 : W + 1] = x.astype(NP_BF16)
    # wt[p, kh*3+kw, o] = weight[o, p, kh, kw]
    wt = np.ascontiguousarray(
        weight.transpose(1, 2, 3, 0).reshape(C_IN, KH * KW, C_OUT).astype(NP_BF16)
    )
    # bz[p, g] = bias[g*128 + p]
    bz = np.ascontiguousarray(bias.reshape(NGRP, 128).T)
    return xp, wt, bz


def kernel(x, weight, bias, trace=False):
    global _NC
    xp, wt, bz = _prep_inputs(x, weight, bias)
    if _NC is None:
        _NC = _build()
    in_maps = [
        {"xp": xp[c * B_LOC : (c + 1) * B_LOC], "wt": wt, "bz": bz}
        for c in range(N_CORES)
    ]
    res = run_bass_kernel_spmd(
        _NC, in_maps, core_ids=list(range(N_CORES)), trace=trace
    )
    outs = [
        r["out"].astype(np.float32).reshape(B_LOC, C_OUT, H, W)
        for r in res.results
    ]
    full = np.concatenate(outs, axis=0)
    if trace:
        return full, res
    return full
